# revision 13
# baseline (speedup 1.0000x reference)
"""Causal attention (B=8, N=4096 flattened 64x64, d=128) on 8 trn2 cores.

Sharding: data-parallel over batch -- core b gets batch element b.

Per-core algorithm (flash-style, transposed orientation):
  inputs per core (host pre-transposed):
    qT [128, 4096] bf16  (c on partitions, query pos on free)
    kT [128, 4096] bf16
    vT [128, 4096] bf16  (k-within-tile on partitions: vT[p, 128j+c] = v[128j+p, c])
  loop q-chunks of 512, k-tiles of 128 (j = 0..4t+3):
    S^T[k, q] = kT_j.T @ qT_chunk          (PE, PSUM, N=512, bf16 moving)
    E = exp(S^T / sqrt(128)) -> bf16       (ScalarE, PSUM->SBUF, groups of 3)
    causal mask on diagonal tiles          (GpSimd affine_select, fill 0)
    O^T += v_j.T @ E_j                     (PE, accumulate in PSUM over j)
    denom[q] += sum_k E_j[k, q]            (DVE adds -> PE ones matmul)
  outputs per core: outT [128, 4096] bf16 (unnormalized O^T), den [1, 4096] f32
  host: out = (outT.astype(f32) / den).T

Scheduling refinements over the first working version:
  * groups are per-chunk with each diagonal tile (dd = 3,2,1,0) placed at a
    GROUP START so the exp activation skips the fully-masked column prefix
    (dd*128 cols) via the existing off0 logic -- saves ~5us of ScalarE.
  * the affine_select only covers the true 128-col diagonal block (the
    masked prefix is never read: S matmul, PV, exp-skip and den adds all
    narrow to [dd*128:]), shrinking gpsimd work ~2.5x.
  * den adds: below-diagonal slices accumulate via ping-pong (never
    in-place) windowed chains -> PE ones-matmul per window; the 4 diagonal
    slices of each chunk form one final window built as copy(d0) then
    narrowed in-place adds, so no stale prefix is ever summed.
  * outT is bf16 (host upcasts): halves the output DMA bytes.

No max-subtraction in softmax: scores are ~N(0,1), exp is safe in fp32 and
softmax is shift-invariant. Masked probabilities are exactly zero.
"""

import math

import ml_dtypes
import numpy as np

import concourse.bacc as bacc
import concourse.mybir as mybir
import concourse.tile as tile
from concourse.bass import ts, ds
from concourse.bass_utils import run_bass_kernel_spmd

P = 128
NSEQ = 4096
QCH = 512              # query positions per chunk
NCH = NSEQ // QCH      # 8 chunks
GROUP = 3              # k-tiles per exp group (3 PSUM banks; x2 buffered)
SCALE = 1.0 / math.sqrt(128.0)
F32 = mybir.dt.float32
I32 = mybir.dt.int32
BF16 = mybir.dt.bfloat16
N_CORES = 8
DEN_WIN = 8            # below-diag tiles per denominator window

# Schraudolph exp on DVE for a few pure below-diagonal groups: the bf16 bit
# pattern of exp(s*SCALE) is approximately s*SCH_K + SCH_B, computed as one
# DVE tensor_scalar (mult, add) with int16 output conversion writing
# straight into the bf16 E tile.  ~3% max rel err on those softmax weights
# only; frees the ScalarE activation queue (the critical engine).
SCH_K = 128.0 * SCALE / math.log(2.0)          # 16.3227
SCH_B = 16256.0 - 5.592                        # min-max-rel bias (floor conv)
# (chunk, pure-below-group index) entries offloaded to DVE
SCH_GROUPS = {(7, 0), (7, 2), (6, 1), (5, 0)}

CHUNK_ORDER = [0, 2, 3, 4, 5, 6, 7, 1]   # start AND end on small chunks
# input pieces (column ranges); piece 0 comes packed in blk0.  q's
# [512:1024) slice (chunk 1, processed last) is fetched dead last.
KV_PIECES = [(0, 512), (512, 1536), (1536, 2560), (2560, 4096)]
Q_PIECES = [(0, 512), (512, 1024), (1024, 1536), (1536, 2560), (2560, 4096)]

_nc_cache = []


def _chunk_seq(t, tail=False):
    """Tile order for chunk t: diagonal tiles (dd=3,2,1,0) lead groups so
    the exp prefix-skip fires; below-diagonal tiles fill the rest.  For the
    tail chunk all diagonals go first so the final group (the serial
    exp->PV->flush tail) has no gpsimd select on its critical path."""
    nj = 4 * (t + 1)
    diags = [4 * t + dd for dd in (3, 2, 1, 0)]
    belows = list(range(4 * t))
    if tail:
        seq = diags + belows
        assert sorted(seq) == list(range(nj))
        return seq
    seq = []
    bi = 0
    for d in diags:
        seq.append(d)
        take = min(GROUP - 1, len(belows) - bi)
        seq.extend(belows[bi:bi + take])
        bi += take
    seq.extend(belows[bi:])
    assert sorted(seq) == list(range(nj))
    return seq


def _build():
    nc = bacc.Bacc("TRN2", target_bir_lowering=False, debug=False,
                   num_devices=N_CORES)
    qT = nc.dram_tensor("qT", [P, NSEQ], BF16, kind="ExternalInput").ap()
    kT = nc.dram_tensor("kT", [P, NSEQ], BF16, kind="ExternalInput").ap()
    vT = nc.dram_tensor("vT", [P, NSEQ], BF16, kind="ExternalInput").ap()
    # ramp-critical first block packed host-side as kT[:512] | qT[:512] |
    # qT[1024:1536] | vT[:512]: 4 KB HBM lines instead of 1 KB, so the
    # early 512 KB moves at ~2-3x the packet rate
    blk0 = nc.dram_tensor("blk0", [P, 4 * 512], BF16,
                          kind="ExternalInput").ap()
    outT = nc.dram_tensor("outT", [P, NSEQ], BF16, kind="ExternalOutput").ap()
    den = nc.dram_tensor("den", [1, NSEQ], F32, kind="ExternalOutput").ap()

    exp_fn = mybir.ActivationFunctionType.Exp
    is_ge = mybir.AluOpType.is_ge

    with tile.TileContext(nc) as tc:
        with (
            tc.tile_pool(name="const", bufs=1) as cpool,
            tc.tile_pool(name="epool", bufs=16) as epool,
            tc.tile_pool(name="qpool", bufs=12) as qpool,
            tc.tile_pool(name="spool", bufs=2) as spool,
            tc.tile_pool(name="ps_s", bufs=2, space="PSUM") as ps_pool,
            tc.tile_pool(name="ps_o", bufs=1, space="PSUM") as po_pool,
            tc.tile_pool(name="ps_d", bufs=1, space="PSUM") as pd_pool,
        ):
            ones_sq = cpool.tile([P, P], BF16)
            nc.gpsimd.memset(ones_sq, 1.0)
            # pre-warm the PE during the input-DMA wait so the HAM clock
            # gate is at 2.4 GHz when real work starts (a PE-idle gap
            # > ~3.4us would re-throttle and the first real groups would
            # run at 1.2 GHz)
            warm_db = pd_pool.tile([P, QCH], F32, tag="db", name="warm")
            for wi in range(72):
                nc.tensor.matmul(warm_db[:, ds(0, 64)], ones_sq,
                                 ones_sq[:, :64], start=True, stop=True)

            # input pieces: separate tiles so their DMAs are independent
            # (a single destination tile serializes the piece DMAs WAW).
            blk0_sb = cpool.tile([P, 4 * 512], BF16, name="blk0")
            kp, qp, vp = {}, {}, {}
            kp[0] = blk0_sb[:, ds(0, 512)]
            qp[0] = blk0_sb[:, ds(512, 512)]
            qp[2] = blk0_sb[:, ds(1024, 512)]
            vp[0] = blk0_sb[:, ds(1536, 512)]
            for pi, (c0, c1) in enumerate(KV_PIECES):
                if pi == 0:
                    continue
                kp[pi] = cpool.tile([P, c1 - c0], BF16, name=f"kp{pi}")
                vp[pi] = cpool.tile([P, c1 - c0], BF16, name=f"vp{pi}")
            for pi, (c0, c1) in enumerate(Q_PIECES):
                if pi in (0, 2):
                    continue
                qp[pi] = cpool.tile([P, c1 - c0], BF16, name=f"qp{pi}")
            # ring discipline: scalar ring carries blk0's first 768 cols
            # (k0 + half of q0), sync ring is FIFO [rest of blk0, then
            # pieces in first-use order]
            nc.scalar.dma_start(blk0_sb[:, ds(0, 768)], blk0[:, ds(0, 768)])
            nc.sync.dma_start(blk0_sb[:, ds(768, 1280)],
                              blk0[:, ds(768, 1280)])
            for tname, pi in (("k", 1), ("v", 1), ("q", 3),
                              ("k", 2), ("v", 2), ("q", 4),
                              ("k", 3), ("v", 3), ("q", 1)):
                tbl = Q_PIECES if tname == "q" else KV_PIECES
                c0, c1 = tbl[pi]
                dst, src = {"q": (qp, qT), "k": (kp, kT),
                            "v": (vp, vT)}[tname]
                nc.sync.dma_start(dst[pi], src[:, ds(c0, c1 - c0)])

            def piece_of(table, col):
                for pi, (c0, c1) in enumerate(table):
                    if c0 <= col < c1:
                        return pi, c0
                raise AssertionError(col)

            def k_tile(j):
                pi, c0 = piece_of(KV_PIECES, j * P)
                return kp[pi][:, ds(j * P - c0, P)]

            def v_tile(j):
                pi, c0 = piece_of(KV_PIECES, j * P)
                return vp[pi][:, ds(j * P - c0, P)]

            def q_chunk(t):
                pi, c0 = piece_of(Q_PIECES, t * QCH)
                return qp[pi][:, ds(t * QCH - c0, QCH)]

            o_tiles, db_tiles = {}, {}

            def emit_pv(ops):
                # deferred PV / den matmuls / chunk flushes for one group
                # (software pipelining: keeps the in-order PE queue's S
                # matmuls ahead of PVs that wait on the gpsimd select)
                for op in ops:
                    kind = op[0]
                    if kind == "pv":
                        _, t, pos, j, nj, e_sb, d = op
                        dd = j - 4 * t
                        off = max(dd, 0) * P
                        nc.tensor.matmul(
                            o_tiles[t][:, ds(off, QCH - off)],
                            v_tile(j),
                            e_sb[:, ds(d * QCH + off, QCH - off)],
                            start=(pos == 0), stop=(pos == nj - 1))
                    elif kind == "dwin":
                        # below-diagonal window: ping-pong add chain then
                        # one ones-matmul (partition reduce) into db
                        _, t, slices, st_, sp_ = op
                        acc = slices[0]
                        for sl in slices[1:]:
                            nxt = qpool.tile([P, QCH], BF16, tag="qacc")
                            nc.vector.tensor_add(nxt, acc, sl)
                            acc = nxt
                        nc.tensor.matmul(db_tiles[t], ones_sq, acc,
                                         start=st_, stop=sp_)
                    elif kind == "ddiag":
                        # diagonal window: copy(d0) + narrowed in-place adds
                        _, t, diag, st_ = op
                        dacc = qpool.tile([P, QCH], BF16, tag="qacc")
                        diag = sorted(diag, key=lambda x: x[1])
                        sl0, o0 = diag[0]
                        assert o0 == 0
                        nc.vector.tensor_copy(dacc, sl0)
                        for sl, off in diag[1:]:
                            nc.vector.tensor_add(
                                dacc[:, ds(off, QCH - off)],
                                dacc[:, ds(off, QCH - off)],
                                sl[:, ds(off, QCH - off)])
                        nc.tensor.matmul(db_tiles[t], ones_sq, dacc,
                                         start=st_, stop=True)
                    else:   # flush: copy chunk outputs + DMA out
                        _, t = op
                        o_ps, db_ps = o_tiles[t], db_tiles[t]
                        out_sb = spool.tile([P, QCH], BF16, tag="osb",
                                            name=f"osb{t}")
                        den_sb = spool.tile([1, QCH], F32, tag="den",
                                            name=f"den{t}")
                        if t == CHUNK_ORDER[-1]:   # tail: split engines
                            nc.scalar.copy(out_sb, o_ps)
                            nc.vector.tensor_copy(den_sb, db_ps[0:1, :])
                            nc.sync.dma_start(outT[:, ts(t, QCH)], out_sb)
                            nc.scalar.dma_start(den[:, ts(t, QCH)], den_sb)
                        else:
                            nc.vector.tensor_copy(out_sb, o_ps)
                            nc.vector.tensor_copy(den_sb, db_ps[0:1, :])
                            nc.sync.dma_start(outT[:, ts(t, QCH)], out_sb)
                            nc.sync.dma_start(den[:, ts(t, QCH)], den_sb)

            # per-chunk den bookkeeping (all DVE work deferred with the ops
            # pipeline so Schraudolph TS ops aren't queued behind it):
            #   below-diagonal e-slices collect into windows of DEN_WIN ->
            #   ping-pong add chain + one ones-matmul per window; diagonal
            #   slices are held to the chunk end and summed as
            #   copy(d0-slice) + narrowed in-place adds (no stale prefix).
            den_state = {}   # t -> dict(pend=[slices], nmm, diag=[(sl,off)])

            def den_below(t, sl, ops):
                st = den_state[t]
                st["pend"].append(sl)
                if len(st["pend"]) >= DEN_WIN:
                    ops.append(("dwin", t, st["pend"], st["nmm"] == 0, False))
                    st["nmm"] += 1
                    st["pend"] = []

            def den_diag_flush(t, ops):
                st = den_state[t]
                if st["pend"]:                 # leftover below-window
                    ops.append(("dwin", t, st["pend"], st["nmm"] == 0, False))
                    st["nmm"] += 1
                    st["pend"] = []
                ops.append(("ddiag", t, st["diag"], st["nmm"] == 0))
                st["nmm"] += 1

            # global tile sequence: per-chunk groups with diagonal tiles
            # leading groups (exp prefix-skip) -- see _chunk_seq.
            entries = []
            for t in CHUNK_ORDER:
                nj = 4 * (t + 1)
                for pos, j in enumerate(_chunk_seq(t, t == CHUNK_ORDER[-1])):
                    entries.append((t, pos, j, nj))

            groups = []
            i = 0
            while i < len(entries):
                t0 = entries[i][0]
                g = [entries[i]]
                while (len(g) < GROUP and i + len(g) < len(entries)
                       and entries[i + len(g)][0] == t0):
                    g.append(entries[i + len(g)])
                groups.append(g)
                i += len(g)

            pv_pending = []
            pure_idx = {}      # chunk -> running index of pure-below groups
            for members in groups:
                gn = len(members)
                s_ps = ps_pool.tile([P, gn * QCH], F32, tag="s",
                                    padded_shape=[P, GROUP * QCH])
                for d, (t, pos, j, nj) in enumerate(members):
                    dd = j - 4 * t
                    off = max(dd, 0) * P   # fully-masked column prefix
                    nc.tensor.matmul(
                        s_ps[:, ds(d * QCH + off, QCH - off)],
                        k_tile(j), q_chunk(t)[:, ds(off, QCH - off)],
                        start=True, stop=True)
                e_sb = epool.tile([P, gn * QCH], BF16, tag="e",
                                  padded_shape=[P, GROUP * QCH])
                # group leader's masked prefix is at the window start ->
                # skip it in the activation
                t0, _, j0_, _ = members[0]
                off0 = max(j0_ - 4 * t0, 0) * P
                pure_below = (gn == GROUP and
                              all(j < 4 * t for t, _, j, _ in members))
                sch = False
                if pure_below:
                    pi_ = pure_idx.get(t0, 0)
                    pure_idx[t0] = pi_ + 1
                    sch = (t0, pi_) in SCH_GROUPS
                if sch:
                    nc.vector.tensor_scalar(
                        e_sb[:, ds(0, gn * QCH)].bitcast(mybir.dt.int16),
                        s_ps[:, ds(0, gn * QCH)], SCH_K, SCH_B,
                        mybir.AluOpType.mult, mybir.AluOpType.add)
                else:
                    nc.scalar.activation(e_sb[:, ds(off0, gn * QCH - off0)],
                                         s_ps[:, ds(off0, gn * QCH - off0)],
                                         exp_fn, scale=SCALE)

                # causal mask: only the true 128-col diagonal block needs
                # the select (prefix cols are never read downstream)
                for d, (t, pos, j, nj) in enumerate(members):
                    dd = j - 4 * t
                    if dd >= 0:
                        reg = e_sb[:, ds(d * QCH + dd * P, P)]
                        nc.gpsimd.affine_select(
                            out=reg, in_=reg, compare_op=is_ge,
                            fill=0.0, base=0, pattern=[[1, P]],
                            channel_multiplier=-1)

                ops = []
                for d, (t, pos, j, nj) in enumerate(members):
                    if pos == 0:
                        o_tiles[t] = po_pool.tile([P, QCH], F32, tag="o",
                                                  name=f"o{t}")
                        db_tiles[t] = pd_pool.tile([P, QCH], F32, tag="db",
                                                   name=f"db{t}")
                        den_state[t] = {"pend": [], "nmm": 0, "diag": []}
                    ops.append(("pv", t, pos, j, nj, e_sb, d))
                    dd = j - 4 * t
                    sl = e_sb[:, ts(d, QCH)]
                    if dd >= 0:
                        den_state[t]["diag"].append((sl, dd * P))
                    else:
                        den_below(t, sl, ops)
                    if pos == nj - 1:
                        den_diag_flush(t, ops)
                        ops.append(("flush", t))

                if len(pv_pending) >= 2:
                    emit_pv(pv_pending.pop(0))
                pv_pending.append(ops)

            for ops in pv_pending:
                emit_pv(ops)

    nc.compile()
    return nc


def _get_nc():
    if not _nc_cache:
        _nc_cache.append(_build())
    return _nc_cache[0]


def _prep(query, key, value):
    B, H, W, C = query.shape
    CV = value.shape[-1]
    n = H * W
    q = (np.asarray(query, np.float32).reshape(B, n, C).transpose(0, 2, 1)
         .astype(ml_dtypes.bfloat16))
    q = np.ascontiguousarray(q)
    k = np.ascontiguousarray(
        np.asarray(key, np.float32).reshape(B, n, C).transpose(0, 2, 1)
        .astype(ml_dtypes.bfloat16))
    # vT[b, p, 128j+c] = v[b, 128j+p, c]: k-within-tile on partitions, so a
    # [128, 128] SBUF slice is directly the PV weight tile, and the HBM
    # lines are long and contiguous (8 KB per partition row)
    v = (np.asarray(value, np.float32).reshape(B, n // P, P, CV)
         .transpose(0, 2, 1, 3).reshape(B, P, n // P * CV)
         .astype(ml_dtypes.bfloat16))
    v = np.ascontiguousarray(v)
    b0 = np.ascontiguousarray(
        np.concatenate([k[:, :, :512], q[:, :, :512], q[:, :, 1024:1536],
                        v[:, :, :512]], axis=2))
    return q, k, v, b0


def kernel(query, key, value):
    B, H, W, C = query.shape
    CV = value.shape[-1]
    n = H * W
    q, k, v, b0 = _prep(query, key, value)

    nc = _get_nc()
    in_maps = [{"qT": q[b], "kT": k[b], "vT": v[b], "blk0": b0[b]}
               for b in range(B)]
    res = run_bass_kernel_spmd(nc, in_maps, core_ids=list(range(N_CORES)))

    out = np.empty((B, n, CV), np.float32)
    for b in range(B):
        oT = np.asarray(res.results[b]["outT"]).astype(np.float32)
        dn = res.results[b]["den"]           # [1, 4096]
        out[b] = (oT / dn).T
    return out.reshape(B, H, W, CV)


# revision 14
# speedup vs baseline: 1.0032x; 1.0032x over previous
"""Causal attention (B=8, N=4096 flattened 64x64, d=128) on 8 trn2 cores.

Sharding: data-parallel over batch -- core b gets batch element b.

Per-core algorithm (flash-style, transposed orientation):
  inputs per core (host pre-transposed):
    qT [128, 4096] bf16  (c on partitions, query pos on free)
    kT [128, 4096] bf16
    vT [128, 4096] bf16  (k-within-tile on partitions: vT[p, 128j+c] = v[128j+p, c])
  loop q-chunks of 512, k-tiles of 128 (j = 0..4t+3):
    S^T[k, q] = kT_j.T @ qT_chunk          (PE, PSUM, N=512, bf16 moving)
    E = exp(S^T / sqrt(128)) -> bf16       (ScalarE, PSUM->SBUF, groups of 3)
    causal mask on diagonal tiles          (GpSimd affine_select, fill 0)
    O^T += v_j.T @ E_j                     (PE, accumulate in PSUM over j)
    denom[q] += sum_k E_j[k, q]            (DVE adds -> PE ones matmul)
  outputs per core: outT [128, 4096] bf16 (unnormalized O^T), den [1, 4096] f32
  host: out = (outT.astype(f32) / den).T

Scheduling refinements over the first working version:
  * groups are per-chunk with each diagonal tile (dd = 3,2,1,0) placed at a
    GROUP START so the exp activation skips the fully-masked column prefix
    (dd*128 cols) via the existing off0 logic -- saves ~5us of ScalarE.
  * the affine_select only covers the true 128-col diagonal block (the
    masked prefix is never read: S matmul, PV, exp-skip and den adds all
    narrow to [dd*128:]), shrinking gpsimd work ~2.5x.
  * den adds: below-diagonal slices accumulate via ping-pong (never
    in-place) windowed chains -> PE ones-matmul per window; the 4 diagonal
    slices of each chunk form one final window built as copy(d0) then
    narrowed in-place adds, so no stale prefix is ever summed.
  * outT is bf16 (host upcasts): halves the output DMA bytes.

No max-subtraction in softmax: scores are ~N(0,1), exp is safe in fp32 and
softmax is shift-invariant. Masked probabilities are exactly zero.
"""

import math

import ml_dtypes
import numpy as np

import concourse.bacc as bacc
import concourse.mybir as mybir
import concourse.tile as tile
from concourse.bass import ts, ds
from concourse.bass_utils import run_bass_kernel_spmd

P = 128
NSEQ = 4096
QCH = 512              # query positions per chunk
NCH = NSEQ // QCH      # 8 chunks
GROUP = 3              # k-tiles per exp group (3 PSUM banks; x2 buffered)
SCALE = 1.0 / math.sqrt(128.0)
F32 = mybir.dt.float32
I32 = mybir.dt.int32
BF16 = mybir.dt.bfloat16
N_CORES = 8
DEN_WIN = 8            # below-diag tiles per denominator window

# Schraudolph exp on DVE for a few pure below-diagonal groups: the bf16 bit
# pattern of exp(s*SCALE) is approximately s*SCH_K + SCH_B, computed as one
# DVE tensor_scalar (mult, add) with int16 output conversion writing
# straight into the bf16 E tile.  ~3% max rel err on those softmax weights
# only; frees the ScalarE activation queue (the critical engine).
SCH_K = 128.0 * SCALE / math.log(2.0)          # 16.3227
SCH_B = 16256.0 - 5.592                        # min-max-rel bias (floor conv)
# (chunk, pure-below-group index) entries offloaded to DVE
SCH_GROUPS = set()

CHUNK_ORDER = [0, 2, 3, 4, 5, 6, 7, 1]   # start AND end on small chunks
# input pieces (column ranges); piece 0 comes packed in blk0.  q's
# [512:1024) slice (chunk 1, processed last) is fetched dead last.
KV_PIECES = [(0, 512), (512, 1536), (1536, 2560), (2560, 4096)]
Q_PIECES = [(0, 512), (512, 1024), (1024, 1536), (1536, 2560), (2560, 4096)]

_nc_cache = []


def _chunk_seq(t, tail=False):
    """Tile order for chunk t: diagonal tiles (dd=3,2,1,0) lead groups so
    the exp prefix-skip fires; below-diagonal tiles fill the rest.  For the
    tail chunk all diagonals go first so the final group (the serial
    exp->PV->flush tail) has no gpsimd select on its critical path."""
    nj = 4 * (t + 1)
    diags = [4 * t + dd for dd in (3, 2, 1, 0)]
    belows = list(range(4 * t))
    if tail:
        seq = diags + belows
        assert sorted(seq) == list(range(nj))
        return seq
    seq = []
    bi = 0
    for d in diags:
        seq.append(d)
        take = min(GROUP - 1, len(belows) - bi)
        seq.extend(belows[bi:bi + take])
        bi += take
    seq.extend(belows[bi:])
    assert sorted(seq) == list(range(nj))
    return seq


def _build():
    nc = bacc.Bacc("TRN2", target_bir_lowering=False, debug=False,
                   num_devices=N_CORES)
    qT = nc.dram_tensor("qT", [P, NSEQ], BF16, kind="ExternalInput").ap()
    kT = nc.dram_tensor("kT", [P, NSEQ], BF16, kind="ExternalInput").ap()
    vT = nc.dram_tensor("vT", [P, NSEQ], BF16, kind="ExternalInput").ap()
    # ramp-critical first block packed host-side as kT[:512] | qT[:512] |
    # qT[1024:1536] | vT[:512]: 4 KB HBM lines instead of 1 KB, so the
    # early 512 KB moves at ~2-3x the packet rate
    blk0 = nc.dram_tensor("blk0", [P, 4 * 512], BF16,
                          kind="ExternalInput").ap()
    outT = nc.dram_tensor("outT", [P, NSEQ], BF16, kind="ExternalOutput").ap()
    den = nc.dram_tensor("den", [1, NSEQ], F32, kind="ExternalOutput").ap()

    exp_fn = mybir.ActivationFunctionType.Exp
    is_ge = mybir.AluOpType.is_ge

    with tile.TileContext(nc) as tc:
        with (
            tc.tile_pool(name="const", bufs=1) as cpool,
            tc.tile_pool(name="epool", bufs=16) as epool,
            tc.tile_pool(name="qpool", bufs=12) as qpool,
            tc.tile_pool(name="spool", bufs=2) as spool,
            tc.tile_pool(name="ps_s", bufs=2, space="PSUM") as ps_pool,
            tc.tile_pool(name="ps_o", bufs=1, space="PSUM") as po_pool,
            tc.tile_pool(name="ps_d", bufs=1, space="PSUM") as pd_pool,
        ):
            ones_sq = cpool.tile([P, P], BF16)
            nc.gpsimd.memset(ones_sq, 1.0)
            # pre-warm the PE during the input-DMA wait so the HAM clock
            # gate is at 2.4 GHz when real work starts (a PE-idle gap
            # > ~3.4us would re-throttle and the first real groups would
            # run at 1.2 GHz)
            warm_db = pd_pool.tile([P, QCH], F32, tag="db", name="warm")
            for wi in range(72):
                nc.tensor.matmul(warm_db[:, ds(0, 64)], ones_sq,
                                 ones_sq[:, :64], start=True, stop=True)

            # input pieces: separate tiles so their DMAs are independent
            # (a single destination tile serializes the piece DMAs WAW).
            blk0_sb = cpool.tile([P, 4 * 512], BF16, name="blk0")
            kp, qp, vp = {}, {}, {}
            kp[0] = blk0_sb[:, ds(0, 512)]
            qp[0] = blk0_sb[:, ds(512, 512)]
            qp[2] = blk0_sb[:, ds(1024, 512)]
            vp[0] = blk0_sb[:, ds(1536, 512)]
            for pi, (c0, c1) in enumerate(KV_PIECES):
                if pi == 0:
                    continue
                kp[pi] = cpool.tile([P, c1 - c0], BF16, name=f"kp{pi}")
                vp[pi] = cpool.tile([P, c1 - c0], BF16, name=f"vp{pi}")
            for pi, (c0, c1) in enumerate(Q_PIECES):
                if pi in (0, 2):
                    continue
                qp[pi] = cpool.tile([P, c1 - c0], BF16, name=f"qp{pi}")
            # ring discipline: scalar ring carries blk0's first 768 cols
            # (k0 + half of q0), sync ring is FIFO [rest of blk0, then
            # pieces in first-use order]
            nc.scalar.dma_start(blk0_sb[:, ds(0, 768)], blk0[:, ds(0, 768)])
            nc.sync.dma_start(blk0_sb[:, ds(768, 1280)],
                              blk0[:, ds(768, 1280)])
            for tname, pi in (("k", 1), ("v", 1), ("q", 3),
                              ("k", 2), ("v", 2), ("q", 4),
                              ("k", 3), ("v", 3), ("q", 1)):
                tbl = Q_PIECES if tname == "q" else KV_PIECES
                c0, c1 = tbl[pi]
                dst, src = {"q": (qp, qT), "k": (kp, kT),
                            "v": (vp, vT)}[tname]
                nc.sync.dma_start(dst[pi], src[:, ds(c0, c1 - c0)])

            def piece_of(table, col):
                for pi, (c0, c1) in enumerate(table):
                    if c0 <= col < c1:
                        return pi, c0
                raise AssertionError(col)

            def k_tile(j):
                pi, c0 = piece_of(KV_PIECES, j * P)
                return kp[pi][:, ds(j * P - c0, P)]

            def v_tile(j):
                pi, c0 = piece_of(KV_PIECES, j * P)
                return vp[pi][:, ds(j * P - c0, P)]

            def q_chunk(t):
                pi, c0 = piece_of(Q_PIECES, t * QCH)
                return qp[pi][:, ds(t * QCH - c0, QCH)]

            o_tiles, db_tiles = {}, {}

            def emit_pv(ops):
                # deferred PV / den matmuls / chunk flushes for one group
                # (software pipelining: keeps the in-order PE queue's S
                # matmuls ahead of PVs that wait on the gpsimd select)
                for op in ops:
                    kind = op[0]
                    if kind == "pv":
                        _, t, pos, j, nj, e_sb, d = op
                        dd = j - 4 * t
                        off = max(dd, 0) * P
                        nc.tensor.matmul(
                            o_tiles[t][:, ds(off, QCH - off)],
                            v_tile(j),
                            e_sb[:, ds(d * QCH + off, QCH - off)],
                            start=(pos == 0), stop=(pos == nj - 1))
                    elif kind == "dwin":
                        # below-diagonal window: ping-pong add chain then
                        # one ones-matmul (partition reduce) into db
                        _, t, slices, st_, sp_ = op
                        acc = slices[0]
                        for sl in slices[1:]:
                            nxt = qpool.tile([P, QCH], BF16, tag="qacc")
                            nc.vector.tensor_add(nxt, acc, sl)
                            acc = nxt
                        nc.tensor.matmul(db_tiles[t], ones_sq, acc,
                                         start=st_, stop=sp_)
                    elif kind == "ddiag":
                        # diagonal window: copy(d0) + narrowed in-place adds
                        _, t, diag, st_ = op
                        dacc = qpool.tile([P, QCH], BF16, tag="qacc")
                        diag = sorted(diag, key=lambda x: x[1])
                        sl0, o0 = diag[0]
                        assert o0 == 0
                        nc.vector.tensor_copy(dacc, sl0)
                        for sl, off in diag[1:]:
                            nc.vector.tensor_add(
                                dacc[:, ds(off, QCH - off)],
                                dacc[:, ds(off, QCH - off)],
                                sl[:, ds(off, QCH - off)])
                        nc.tensor.matmul(db_tiles[t], ones_sq, dacc,
                                         start=st_, stop=True)
                    else:   # flush: copy chunk outputs + DMA out
                        _, t = op
                        o_ps, db_ps = o_tiles[t], db_tiles[t]
                        out_sb = spool.tile([P, QCH], BF16, tag="osb",
                                            name=f"osb{t}")
                        den_sb = spool.tile([1, QCH], F32, tag="den",
                                            name=f"den{t}")
                        if t == CHUNK_ORDER[-1]:   # tail: split engines
                            nc.scalar.copy(out_sb, o_ps)
                            nc.vector.tensor_copy(den_sb, db_ps[0:1, :])
                            nc.sync.dma_start(outT[:, ts(t, QCH)], out_sb)
                            nc.scalar.dma_start(den[:, ts(t, QCH)], den_sb)
                        else:
                            nc.vector.tensor_copy(out_sb, o_ps)
                            nc.vector.tensor_copy(den_sb, db_ps[0:1, :])
                            nc.sync.dma_start(outT[:, ts(t, QCH)], out_sb)
                            nc.sync.dma_start(den[:, ts(t, QCH)], den_sb)

            # per-chunk den bookkeeping (all DVE work deferred with the ops
            # pipeline so Schraudolph TS ops aren't queued behind it):
            #   below-diagonal e-slices collect into windows of DEN_WIN ->
            #   ping-pong add chain + one ones-matmul per window; diagonal
            #   slices are held to the chunk end and summed as
            #   copy(d0-slice) + narrowed in-place adds (no stale prefix).
            den_state = {}   # t -> dict(pend=[slices], nmm, diag=[(sl,off)])

            def den_below(t, sl, ops):
                st = den_state[t]
                st["pend"].append(sl)
                if len(st["pend"]) >= DEN_WIN:
                    ops.append(("dwin", t, st["pend"], st["nmm"] == 0, False))
                    st["nmm"] += 1
                    st["pend"] = []

            def den_diag_flush(t, ops):
                st = den_state[t]
                if st["pend"]:                 # leftover below-window
                    ops.append(("dwin", t, st["pend"], st["nmm"] == 0, False))
                    st["nmm"] += 1
                    st["pend"] = []
                ops.append(("ddiag", t, st["diag"], st["nmm"] == 0))
                st["nmm"] += 1

            # global tile sequence: per-chunk groups with diagonal tiles
            # leading groups (exp prefix-skip) -- see _chunk_seq.
            entries = []
            for t in CHUNK_ORDER:
                nj = 4 * (t + 1)
                for pos, j in enumerate(_chunk_seq(t, t == CHUNK_ORDER[-1])):
                    entries.append((t, pos, j, nj))

            groups = []
            i = 0
            while i < len(entries):
                t0 = entries[i][0]
                g = [entries[i]]
                while (len(g) < GROUP and i + len(g) < len(entries)
                       and entries[i + len(g)][0] == t0):
                    g.append(entries[i + len(g)])
                groups.append(g)
                i += len(g)

            pv_pending = []
            pure_idx = {}      # chunk -> running index of pure-below groups
            for members in groups:
                gn = len(members)
                s_ps = ps_pool.tile([P, gn * QCH], F32, tag="s",
                                    padded_shape=[P, GROUP * QCH])
                for d, (t, pos, j, nj) in enumerate(members):
                    dd = j - 4 * t
                    off = max(dd, 0) * P   # fully-masked column prefix
                    nc.tensor.matmul(
                        s_ps[:, ds(d * QCH + off, QCH - off)],
                        k_tile(j), q_chunk(t)[:, ds(off, QCH - off)],
                        start=True, stop=True)
                e_sb = epool.tile([P, gn * QCH], BF16, tag="e",
                                  padded_shape=[P, GROUP * QCH])
                # group leader's masked prefix is at the window start ->
                # skip it in the activation
                t0, _, j0_, _ = members[0]
                off0 = max(j0_ - 4 * t0, 0) * P
                pure_below = (gn == GROUP and
                              all(j < 4 * t for t, _, j, _ in members))
                sch = False
                if pure_below:
                    pi_ = pure_idx.get(t0, 0)
                    pure_idx[t0] = pi_ + 1
                    sch = (t0, pi_) in SCH_GROUPS
                if sch:
                    nc.vector.tensor_scalar(
                        e_sb[:, ds(0, gn * QCH)].bitcast(mybir.dt.int16),
                        s_ps[:, ds(0, gn * QCH)], SCH_K, SCH_B,
                        mybir.AluOpType.mult, mybir.AluOpType.add)
                else:
                    nc.scalar.activation(e_sb[:, ds(off0, gn * QCH - off0)],
                                         s_ps[:, ds(off0, gn * QCH - off0)],
                                         exp_fn, scale=SCALE)

                # causal mask: only the true 128-col diagonal block needs
                # the select (prefix cols are never read downstream)
                for d, (t, pos, j, nj) in enumerate(members):
                    dd = j - 4 * t
                    if dd >= 0:
                        reg = e_sb[:, ds(d * QCH + dd * P, P)]
                        nc.gpsimd.affine_select(
                            out=reg, in_=reg, compare_op=is_ge,
                            fill=0.0, base=0, pattern=[[1, P]],
                            channel_multiplier=-1)

                ops = []
                for d, (t, pos, j, nj) in enumerate(members):
                    if pos == 0:
                        o_tiles[t] = po_pool.tile([P, QCH], F32, tag="o",
                                                  name=f"o{t}")
                        db_tiles[t] = pd_pool.tile([P, QCH], F32, tag="db",
                                                   name=f"db{t}")
                        den_state[t] = {"pend": [], "nmm": 0, "diag": []}
                    ops.append(("pv", t, pos, j, nj, e_sb, d))
                    dd = j - 4 * t
                    sl = e_sb[:, ts(d, QCH)]
                    if dd >= 0:
                        den_state[t]["diag"].append((sl, dd * P))
                    else:
                        den_below(t, sl, ops)
                    if pos == nj - 1:
                        den_diag_flush(t, ops)
                        ops.append(("flush", t))

                if len(pv_pending) >= 2:
                    emit_pv(pv_pending.pop(0))
                pv_pending.append(ops)

            for ops in pv_pending:
                emit_pv(ops)

    nc.compile()
    return nc


def _get_nc():
    if not _nc_cache:
        _nc_cache.append(_build())
    return _nc_cache[0]


def _prep(query, key, value):
    B, H, W, C = query.shape
    CV = value.shape[-1]
    n = H * W
    q = (np.asarray(query, np.float32).reshape(B, n, C).transpose(0, 2, 1)
         .astype(ml_dtypes.bfloat16))
    q = np.ascontiguousarray(q)
    k = np.ascontiguousarray(
        np.asarray(key, np.float32).reshape(B, n, C).transpose(0, 2, 1)
        .astype(ml_dtypes.bfloat16))
    # vT[b, p, 128j+c] = v[b, 128j+p, c]: k-within-tile on partitions, so a
    # [128, 128] SBUF slice is directly the PV weight tile, and the HBM
    # lines are long and contiguous (8 KB per partition row)
    v = (np.asarray(value, np.float32).reshape(B, n // P, P, CV)
         .transpose(0, 2, 1, 3).reshape(B, P, n // P * CV)
         .astype(ml_dtypes.bfloat16))
    v = np.ascontiguousarray(v)
    b0 = np.ascontiguousarray(
        np.concatenate([k[:, :, :512], q[:, :, :512], q[:, :, 1024:1536],
                        v[:, :, :512]], axis=2))
    return q, k, v, b0


def kernel(query, key, value):
    B, H, W, C = query.shape
    CV = value.shape[-1]
    n = H * W
    q, k, v, b0 = _prep(query, key, value)

    nc = _get_nc()
    in_maps = [{"qT": q[b], "kT": k[b], "vT": v[b], "blk0": b0[b]}
               for b in range(B)]
    res = run_bass_kernel_spmd(nc, in_maps, core_ids=list(range(N_CORES)))

    out = np.empty((B, n, CV), np.float32)
    for b in range(B):
        oT = np.asarray(res.results[b]["outT"]).astype(np.float32)
        dn = res.results[b]["den"]           # [1, 4096]
        out[b] = (oT / dn).T
    return out.reshape(B, H, W, CV)


# revision 17
# speedup vs baseline: 1.0079x; 1.0047x over previous
"""Causal attention (B=8, N=4096 flattened 64x64, d=128) on 8 trn2 cores.

Sharding: data-parallel over batch -- core b gets batch element b.

Per-core algorithm (flash-style, transposed orientation):
  inputs per core (host pre-transposed):
    qT [128, 4096] bf16  (c on partitions, query pos on free)
    kT [128, 4096] bf16
    vT [128, 4096] bf16  (k-within-tile on partitions: vT[p, 128j+c] = v[128j+p, c])
  loop q-chunks of 512, k-tiles of 128 (j = 0..4t+3):
    S^T[k, q] = kT_j.T @ qT_chunk          (PE, PSUM, N=512, bf16 moving)
    E = exp(S^T / sqrt(128)) -> bf16       (ScalarE, PSUM->SBUF, groups of 3)
    causal mask on diagonal tiles          (GpSimd affine_select, fill 0)
    O^T += v_j.T @ E_j                     (PE, accumulate in PSUM over j)
    denom[q] += sum_k E_j[k, q]            (DVE adds -> PE ones matmul)
  outputs per core: outT [128, 4096] bf16 (unnormalized O^T), den [1, 4096] f32
  host: out = (outT.astype(f32) / den).T

Scheduling refinements over the first working version:
  * groups are per-chunk with each diagonal tile (dd = 3,2,1,0) placed at a
    GROUP START so the exp activation skips the fully-masked column prefix
    (dd*128 cols) via the existing off0 logic -- saves ~5us of ScalarE.
  * the affine_select only covers the true 128-col diagonal block (the
    masked prefix is never read: S matmul, PV, exp-skip and den adds all
    narrow to [dd*128:]), shrinking gpsimd work ~2.5x.
  * den adds: below-diagonal slices accumulate via ping-pong (never
    in-place) windowed chains -> PE ones-matmul per window; the 4 diagonal
    slices of each chunk form one final window built as copy(d0) then
    narrowed in-place adds, so no stale prefix is ever summed.
  * outT is bf16 (host upcasts): halves the output DMA bytes.

No max-subtraction in softmax: scores are ~N(0,1), exp is safe in fp32 and
softmax is shift-invariant. Masked probabilities are exactly zero.
"""

import math

import ml_dtypes
import numpy as np

import concourse.bacc as bacc
import concourse.mybir as mybir
import concourse.tile as tile
from concourse.bass import ts, ds
from concourse.bass_utils import run_bass_kernel_spmd

P = 128
NSEQ = 4096
QCH = 512              # query positions per chunk
NCH = NSEQ // QCH      # 8 chunks
GROUP = 3              # k-tiles per exp group (3 PSUM banks; x2 buffered)
SCALE = 1.0 / math.sqrt(128.0)
F32 = mybir.dt.float32
I32 = mybir.dt.int32
BF16 = mybir.dt.bfloat16
N_CORES = 8
DEN_WIN = 6            # below-diag tiles per denominator window

# Schraudolph exp on DVE for a few pure below-diagonal groups: the bf16 bit
# pattern of exp(s*SCALE) is approximately s*SCH_K + SCH_B, computed as one
# DVE tensor_scalar (mult, add) with int16 output conversion writing
# straight into the bf16 E tile.  ~3% max rel err on those softmax weights
# only; frees the ScalarE activation queue (the critical engine).
SCH_K = 128.0 * SCALE / math.log(2.0)          # 16.3227
SCH_B = 16256.0 - 5.592                        # min-max-rel bias (floor conv)
# (chunk, pure-below-group index) entries offloaded to DVE
SCH_GROUPS = {(7, 0), (7, 2), (6, 1), (5, 0)}

CHUNK_ORDER = [0, 2, 3, 4, 5, 6, 7, 1]   # start AND end on small chunks
# input pieces (column ranges); piece 0 comes packed in blk0.  q's
# [512:1024) slice (chunk 1, processed last) is fetched dead last.
KV_PIECES = [(0, 512), (512, 1536), (1536, 2560), (2560, 4096)]
Q_PIECES = [(0, 512), (512, 1024), (1024, 1536), (1536, 2560), (2560, 4096)]

_nc_cache = []


def _chunk_seq(t, tail=False):
    """Tile order for chunk t: diagonal tiles (dd=3,2,1,0) lead groups so
    the exp prefix-skip fires; below-diagonal tiles fill the rest.  For the
    tail chunk all diagonals go first so the final group (the serial
    exp->PV->flush tail) has no gpsimd select on its critical path."""
    nj = 4 * (t + 1)
    diags = [4 * t + dd for dd in (3, 2, 1, 0)]
    belows = list(range(4 * t))
    if tail:
        seq = diags + belows
        assert sorted(seq) == list(range(nj))
        return seq
    seq = []
    bi = 0
    for d in diags:
        seq.append(d)
        take = min(GROUP - 1, len(belows) - bi)
        seq.extend(belows[bi:bi + take])
        bi += take
    seq.extend(belows[bi:])
    assert sorted(seq) == list(range(nj))
    return seq


def _build():
    nc = bacc.Bacc("TRN2", target_bir_lowering=False, debug=False,
                   num_devices=N_CORES)
    qT = nc.dram_tensor("qT", [P, NSEQ], BF16, kind="ExternalInput").ap()
    kT = nc.dram_tensor("kT", [P, NSEQ], BF16, kind="ExternalInput").ap()
    vT = nc.dram_tensor("vT", [P, NSEQ], BF16, kind="ExternalInput").ap()
    # ramp-critical first block packed host-side as kT[:512] | qT[:512] |
    # qT[1024:1536] | vT[:512]: 4 KB HBM lines instead of 1 KB, so the
    # early 512 KB moves at ~2-3x the packet rate
    blk0 = nc.dram_tensor("blk0", [P, 4 * 512], BF16,
                          kind="ExternalInput").ap()
    outT = nc.dram_tensor("outT", [P, NSEQ], BF16, kind="ExternalOutput").ap()
    den = nc.dram_tensor("den", [1, NSEQ], F32, kind="ExternalOutput").ap()

    exp_fn = mybir.ActivationFunctionType.Exp
    is_ge = mybir.AluOpType.is_ge

    with tile.TileContext(nc) as tc:
        with (
            tc.tile_pool(name="const", bufs=1) as cpool,
            tc.tile_pool(name="epool", bufs=16) as epool,
            tc.tile_pool(name="qpool", bufs=12) as qpool,
            tc.tile_pool(name="spool", bufs=2) as spool,
            tc.tile_pool(name="ps_s", bufs=2, space="PSUM") as ps_pool,
            tc.tile_pool(name="ps_o", bufs=1, space="PSUM") as po_pool,
            tc.tile_pool(name="ps_d", bufs=1, space="PSUM") as pd_pool,
        ):
            ones_sq = cpool.tile([P, P], BF16)
            nc.gpsimd.memset(ones_sq, 1.0)
            # pre-warm the PE during the input-DMA wait so the HAM clock
            # gate is at 2.4 GHz when real work starts (a PE-idle gap
            # > ~3.4us would re-throttle and the first real groups would
            # run at 1.2 GHz)
            warm_db = pd_pool.tile([P, QCH], F32, tag="db", name="warm")
            for wi in range(72):
                nc.tensor.matmul(warm_db[:, ds(0, 64)], ones_sq,
                                 ones_sq[:, :64], start=True, stop=True)

            # input pieces: separate tiles so their DMAs are independent
            # (a single destination tile serializes the piece DMAs WAW).
            blk0_sb = cpool.tile([P, 4 * 512], BF16, name="blk0")
            kp, qp, vp = {}, {}, {}
            kp[0] = blk0_sb[:, ds(0, 512)]
            qp[0] = blk0_sb[:, ds(512, 512)]
            qp[2] = blk0_sb[:, ds(1024, 512)]
            vp[0] = blk0_sb[:, ds(1536, 512)]
            for pi, (c0, c1) in enumerate(KV_PIECES):
                if pi == 0:
                    continue
                kp[pi] = cpool.tile([P, c1 - c0], BF16, name=f"kp{pi}")
                vp[pi] = cpool.tile([P, c1 - c0], BF16, name=f"vp{pi}")
            for pi, (c0, c1) in enumerate(Q_PIECES):
                if pi in (0, 2):
                    continue
                qp[pi] = cpool.tile([P, c1 - c0], BF16, name=f"qp{pi}")
            # ring discipline: scalar ring carries blk0's first 768 cols
            # (k0 + half of q0), sync ring is FIFO [rest of blk0, then
            # pieces in first-use order]
            nc.scalar.dma_start(blk0_sb[:, ds(0, 768)], blk0[:, ds(0, 768)])
            nc.sync.dma_start(blk0_sb[:, ds(768, 1280)],
                              blk0[:, ds(768, 1280)])
            for tname, pi in (("k", 1), ("v", 1), ("q", 3),
                              ("k", 2), ("v", 2), ("q", 4),
                              ("k", 3), ("v", 3), ("q", 1)):
                tbl = Q_PIECES if tname == "q" else KV_PIECES
                c0, c1 = tbl[pi]
                dst, src = {"q": (qp, qT), "k": (kp, kT),
                            "v": (vp, vT)}[tname]
                nc.sync.dma_start(dst[pi], src[:, ds(c0, c1 - c0)])

            def piece_of(table, col):
                for pi, (c0, c1) in enumerate(table):
                    if c0 <= col < c1:
                        return pi, c0
                raise AssertionError(col)

            def k_tile(j):
                pi, c0 = piece_of(KV_PIECES, j * P)
                return kp[pi][:, ds(j * P - c0, P)]

            def v_tile(j):
                pi, c0 = piece_of(KV_PIECES, j * P)
                return vp[pi][:, ds(j * P - c0, P)]

            def q_chunk(t):
                pi, c0 = piece_of(Q_PIECES, t * QCH)
                return qp[pi][:, ds(t * QCH - c0, QCH)]

            o_tiles, db_tiles = {}, {}

            def emit_pv(ops):
                # deferred PV / den matmuls / chunk flushes for one group
                # (software pipelining: keeps the in-order PE queue's S
                # matmuls ahead of PVs that wait on the gpsimd select)
                for op in ops:
                    kind = op[0]
                    if kind == "pv":
                        _, t, pos, j, nj, e_sb, d = op
                        dd = j - 4 * t
                        off = max(dd, 0) * P
                        nc.tensor.matmul(
                            o_tiles[t][:, ds(off, QCH - off)],
                            v_tile(j),
                            e_sb[:, ds(d * QCH + off, QCH - off)],
                            start=(pos == 0), stop=(pos == nj - 1))
                    elif kind == "dwin":
                        # below-diagonal window: ping-pong add chain then
                        # one ones-matmul (partition reduce) into db
                        _, t, slices, st_, sp_ = op
                        acc = slices[0]
                        for sl in slices[1:]:
                            nxt = qpool.tile([P, QCH], BF16, tag="qacc")
                            nc.vector.tensor_add(nxt, acc, sl)
                            acc = nxt
                        nc.tensor.matmul(db_tiles[t], ones_sq, acc,
                                         start=st_, stop=sp_)
                    elif kind == "ddiag":
                        # diagonal window: copy(d0) + narrowed in-place adds
                        _, t, diag, st_ = op
                        dacc = qpool.tile([P, QCH], BF16, tag="qacc")
                        diag = sorted(diag, key=lambda x: x[1])
                        sl0, o0 = diag[0]
                        assert o0 == 0
                        nc.vector.tensor_copy(dacc, sl0)
                        for sl, off in diag[1:]:
                            nc.vector.tensor_add(
                                dacc[:, ds(off, QCH - off)],
                                dacc[:, ds(off, QCH - off)],
                                sl[:, ds(off, QCH - off)])
                        nc.tensor.matmul(db_tiles[t], ones_sq, dacc,
                                         start=st_, stop=True)
                    else:   # flush: copy chunk outputs + DMA out
                        _, t = op
                        o_ps, db_ps = o_tiles[t], db_tiles[t]
                        out_sb = spool.tile([P, QCH], BF16, tag="osb",
                                            name=f"osb{t}")
                        den_sb = spool.tile([1, QCH], F32, tag="den",
                                            name=f"den{t}")
                        if t == CHUNK_ORDER[-1]:   # tail: split engines
                            nc.scalar.copy(out_sb, o_ps)
                            nc.vector.tensor_copy(den_sb, db_ps[0:1, :])
                            nc.sync.dma_start(outT[:, ts(t, QCH)], out_sb)
                            nc.scalar.dma_start(den[:, ts(t, QCH)], den_sb)
                        else:
                            nc.vector.tensor_copy(out_sb, o_ps)
                            nc.vector.tensor_copy(den_sb, db_ps[0:1, :])
                            nc.sync.dma_start(outT[:, ts(t, QCH)], out_sb)
                            nc.sync.dma_start(den[:, ts(t, QCH)], den_sb)

            # per-chunk den bookkeeping (all DVE work deferred with the ops
            # pipeline so Schraudolph TS ops aren't queued behind it):
            #   below-diagonal e-slices collect into windows of DEN_WIN ->
            #   ping-pong add chain + one ones-matmul per window; diagonal
            #   slices are held to the chunk end and summed as
            #   copy(d0-slice) + narrowed in-place adds (no stale prefix).
            den_state = {}   # t -> dict(pend=[slices], nmm, diag=[(sl,off)])

            def den_below(t, sl, ops):
                st = den_state[t]
                st["pend"].append(sl)
                if len(st["pend"]) >= DEN_WIN:
                    ops.append(("dwin", t, st["pend"], st["nmm"] == 0, False))
                    st["nmm"] += 1
                    st["pend"] = []

            def den_diag_flush(t, ops):
                st = den_state[t]
                if st["pend"]:                 # leftover below-window
                    ops.append(("dwin", t, st["pend"], st["nmm"] == 0, False))
                    st["nmm"] += 1
                    st["pend"] = []
                ops.append(("ddiag", t, st["diag"], st["nmm"] == 0))
                st["nmm"] += 1

            # global tile sequence: per-chunk groups with diagonal tiles
            # leading groups (exp prefix-skip) -- see _chunk_seq.
            entries = []
            for t in CHUNK_ORDER:
                nj = 4 * (t + 1)
                for pos, j in enumerate(_chunk_seq(t)):
                    entries.append((t, pos, j, nj))

            groups = []
            i = 0
            while i < len(entries):
                t0 = entries[i][0]
                g = [entries[i]]
                while (len(g) < GROUP and i + len(g) < len(entries)
                       and entries[i + len(g)][0] == t0):
                    g.append(entries[i + len(g)])
                groups.append(g)
                i += len(g)

            pv_pending = []
            pure_idx = {}      # chunk -> running index of pure-below groups
            for members in groups:
                gn = len(members)
                s_ps = ps_pool.tile([P, gn * QCH], F32, tag="s",
                                    padded_shape=[P, GROUP * QCH])
                for d, (t, pos, j, nj) in enumerate(members):
                    dd = j - 4 * t
                    off = max(dd, 0) * P   # fully-masked column prefix
                    nc.tensor.matmul(
                        s_ps[:, ds(d * QCH + off, QCH - off)],
                        k_tile(j), q_chunk(t)[:, ds(off, QCH - off)],
                        start=True, stop=True)
                e_sb = epool.tile([P, gn * QCH], BF16, tag="e",
                                  padded_shape=[P, GROUP * QCH])
                # group leader's masked prefix is at the window start ->
                # skip it in the activation
                t0, _, j0_, _ = members[0]
                off0 = max(j0_ - 4 * t0, 0) * P
                pure_below = (gn == GROUP and
                              all(j < 4 * t for t, _, j, _ in members))
                sch = False
                if pure_below:
                    pi_ = pure_idx.get(t0, 0)
                    pure_idx[t0] = pi_ + 1
                    sch = (t0, pi_) in SCH_GROUPS
                if sch:
                    nc.vector.tensor_scalar(
                        e_sb[:, ds(0, gn * QCH)].bitcast(mybir.dt.int16),
                        s_ps[:, ds(0, gn * QCH)], SCH_K, SCH_B,
                        mybir.AluOpType.mult, mybir.AluOpType.add)
                else:
                    nc.scalar.activation(e_sb[:, ds(off0, gn * QCH - off0)],
                                         s_ps[:, ds(off0, gn * QCH - off0)],
                                         exp_fn, scale=SCALE)

                # causal mask: only the true 128-col diagonal block needs
                # the select (prefix cols are never read downstream)
                for d, (t, pos, j, nj) in enumerate(members):
                    dd = j - 4 * t
                    if dd >= 0:
                        reg = e_sb[:, ds(d * QCH + dd * P, P)]
                        nc.gpsimd.affine_select(
                            out=reg, in_=reg, compare_op=is_ge,
                            fill=0.0, base=0, pattern=[[1, P]],
                            channel_multiplier=-1)

                ops = []
                for d, (t, pos, j, nj) in enumerate(members):
                    if pos == 0:
                        o_tiles[t] = po_pool.tile([P, QCH], F32, tag="o",
                                                  name=f"o{t}")
                        db_tiles[t] = pd_pool.tile([P, QCH], F32, tag="db",
                                                   name=f"db{t}")
                        den_state[t] = {"pend": [], "nmm": 0, "diag": []}
                    ops.append(("pv", t, pos, j, nj, e_sb, d))
                    dd = j - 4 * t
                    sl = e_sb[:, ts(d, QCH)]
                    if dd >= 0:
                        den_state[t]["diag"].append((sl, dd * P))
                    else:
                        den_below(t, sl, ops)
                    if pos == nj - 1:
                        den_diag_flush(t, ops)
                        ops.append(("flush", t))

                if len(pv_pending) >= 2:
                    emit_pv(pv_pending.pop(0))
                pv_pending.append(ops)

            for ops in pv_pending:
                emit_pv(ops)

    nc.compile()
    return nc


def _get_nc():
    if not _nc_cache:
        _nc_cache.append(_build())
    return _nc_cache[0]


def _prep(query, key, value):
    B, H, W, C = query.shape
    CV = value.shape[-1]
    n = H * W
    q = (np.asarray(query, np.float32).reshape(B, n, C).transpose(0, 2, 1)
         .astype(ml_dtypes.bfloat16))
    q = np.ascontiguousarray(q)
    k = np.ascontiguousarray(
        np.asarray(key, np.float32).reshape(B, n, C).transpose(0, 2, 1)
        .astype(ml_dtypes.bfloat16))
    # vT[b, p, 128j+c] = v[b, 128j+p, c]: k-within-tile on partitions, so a
    # [128, 128] SBUF slice is directly the PV weight tile, and the HBM
    # lines are long and contiguous (8 KB per partition row)
    v = (np.asarray(value, np.float32).reshape(B, n // P, P, CV)
         .transpose(0, 2, 1, 3).reshape(B, P, n // P * CV)
         .astype(ml_dtypes.bfloat16))
    v = np.ascontiguousarray(v)
    b0 = np.ascontiguousarray(
        np.concatenate([k[:, :, :512], q[:, :, :512], q[:, :, 1024:1536],
                        v[:, :, :512]], axis=2))
    return q, k, v, b0


def kernel(query, key, value):
    B, H, W, C = query.shape
    CV = value.shape[-1]
    n = H * W
    q, k, v, b0 = _prep(query, key, value)

    nc = _get_nc()
    in_maps = [{"qT": q[b], "kT": k[b], "vT": v[b], "blk0": b0[b]}
               for b in range(B)]
    res = run_bass_kernel_spmd(nc, in_maps, core_ids=list(range(N_CORES)))

    out = np.empty((B, n, CV), np.float32)
    for b in range(B):
        oT = np.asarray(res.results[b]["outT"]).astype(np.float32)
        dn = res.results[b]["den"]           # [1, 4096]
        out[b] = (oT / dn).T
    return out.reshape(B, H, W, CV)


# revision 18
# speedup vs baseline: 1.0229x; 1.0148x over previous
"""Causal attention (B=8, N=4096 flattened 64x64, d=128) on 8 trn2 cores.

Sharding: data-parallel over batch -- core b gets batch element b.

Per-core algorithm (flash-style, transposed orientation):
  inputs per core (host pre-transposed):
    qT [128, 4096] bf16  (c on partitions, query pos on free)
    kT [128, 4096] bf16
    vT [128, 4096] bf16  (k-within-tile on partitions: vT[p, 128j+c] = v[128j+p, c])
  loop q-chunks of 512, k-tiles of 128 (j = 0..4t+3):
    S^T[k, q] = kT_j.T @ qT_chunk          (PE, PSUM, N=512, bf16 moving)
    E = exp(S^T / sqrt(128)) -> bf16       (ScalarE, PSUM->SBUF, groups of 3)
    causal mask on diagonal tiles          (GpSimd affine_select, fill 0)
    O^T += v_j.T @ E_j                     (PE, accumulate in PSUM over j)
    denom[q] += sum_k E_j[k, q]            (DVE adds -> PE ones matmul)
  outputs per core: outT [128, 4096] bf16 (unnormalized O^T), den [1, 4096] f32
  host: out = (outT.astype(f32) / den).T

Scheduling refinements over the first working version:
  * groups are per-chunk with each diagonal tile (dd = 3,2,1,0) placed at a
    GROUP START so the exp activation skips the fully-masked column prefix
    (dd*128 cols) via the existing off0 logic -- saves ~5us of ScalarE.
  * the affine_select only covers the true 128-col diagonal block (the
    masked prefix is never read: S matmul, PV, exp-skip and den adds all
    narrow to [dd*128:]), shrinking gpsimd work ~2.5x.
  * den adds: below-diagonal slices accumulate via ping-pong (never
    in-place) windowed chains -> PE ones-matmul per window; the 4 diagonal
    slices of each chunk form one final window built as copy(d0) then
    narrowed in-place adds, so no stale prefix is ever summed.
  * outT is bf16 (host upcasts): halves the output DMA bytes.

No max-subtraction in softmax: scores are ~N(0,1), exp is safe in fp32 and
softmax is shift-invariant. Masked probabilities are exactly zero.
"""

import math

import ml_dtypes
import numpy as np

import concourse.bacc as bacc
import concourse.mybir as mybir
import concourse.tile as tile
from concourse.bass import ts, ds
from concourse.bass_utils import run_bass_kernel_spmd

P = 128
NSEQ = 4096
QCH = 512              # query positions per chunk
NCH = NSEQ // QCH      # 8 chunks
GROUP = 3              # k-tiles per exp group (3 PSUM banks; x2 buffered)
SCALE = 1.0 / math.sqrt(128.0)
F32 = mybir.dt.float32
I32 = mybir.dt.int32
BF16 = mybir.dt.bfloat16
N_CORES = 8
DEN_WIN = 16           # below-diag tiles per denominator window

# Schraudolph exp on DVE for a few pure below-diagonal groups: the bf16 bit
# pattern of exp(s*SCALE) is approximately s*SCH_K + SCH_B, computed as one
# DVE tensor_scalar (mult, add) with int16 output conversion writing
# straight into the bf16 E tile.  ~3% max rel err on those softmax weights
# only; frees the ScalarE activation queue (the critical engine).
SCH_K = 128.0 * SCALE / math.log(2.0)          # 16.3227
SCH_B = 16256.0 - 5.592                        # min-max-rel bias (floor conv)
# (chunk, pure-below-group index) entries offloaded to DVE
SCH_GROUPS = set()

CHUNK_ORDER = [0, 2, 3, 4, 5, 6, 7, 1]   # start AND end on small chunks
# input pieces (column ranges); piece 0 comes packed in blk0.  q's
# [512:1024) slice (chunk 1, processed last) is fetched dead last.
KV_PIECES = [(0, 512), (512, 1536), (1536, 2560), (2560, 4096)]
Q_PIECES = [(0, 512), (512, 1024), (1024, 1536), (1536, 2560), (2560, 4096)]

_nc_cache = []


def _chunk_seq(t, tail=False):
    """Tile order for chunk t: diagonal tiles (dd=3,2,1,0) lead groups so
    the exp prefix-skip fires; below-diagonal tiles fill the rest.  For the
    tail chunk all diagonals go first so the final group (the serial
    exp->PV->flush tail) has no gpsimd select on its critical path."""
    nj = 4 * (t + 1)
    diags = [4 * t + dd for dd in (3, 2, 1, 0)]
    belows = list(range(4 * t))
    if tail:
        seq = diags + belows
        assert sorted(seq) == list(range(nj))
        return seq
    seq = []
    bi = 0
    for d in diags:
        seq.append(d)
        take = min(GROUP - 1, len(belows) - bi)
        seq.extend(belows[bi:bi + take])
        bi += take
    seq.extend(belows[bi:])
    assert sorted(seq) == list(range(nj))
    return seq


def _build():
    nc = bacc.Bacc("TRN2", target_bir_lowering=False, debug=False,
                   num_devices=N_CORES)
    qT = nc.dram_tensor("qT", [P, NSEQ], BF16, kind="ExternalInput").ap()
    kT = nc.dram_tensor("kT", [P, NSEQ], BF16, kind="ExternalInput").ap()
    vT = nc.dram_tensor("vT", [P, NSEQ], BF16, kind="ExternalInput").ap()
    # ramp-critical first block packed host-side as kT[:512] | qT[:512] |
    # qT[1024:1536] | vT[:512]: 4 KB HBM lines instead of 1 KB, so the
    # early 512 KB moves at ~2-3x the packet rate
    blk0 = nc.dram_tensor("blk0", [P, 4 * 512], BF16,
                          kind="ExternalInput").ap()
    outT = nc.dram_tensor("outT", [P, NSEQ], BF16, kind="ExternalOutput").ap()
    den = nc.dram_tensor("den", [1, NSEQ], F32, kind="ExternalOutput").ap()

    exp_fn = mybir.ActivationFunctionType.Exp
    is_ge = mybir.AluOpType.is_ge

    with tile.TileContext(nc) as tc:
        with (
            tc.tile_pool(name="const", bufs=1) as cpool,
            tc.tile_pool(name="epool", bufs=16) as epool,
            tc.tile_pool(name="qpool", bufs=12) as qpool,
            tc.tile_pool(name="spool", bufs=2) as spool,
            tc.tile_pool(name="ps_s", bufs=2, space="PSUM") as ps_pool,
            tc.tile_pool(name="ps_o", bufs=1, space="PSUM") as po_pool,
            tc.tile_pool(name="ps_d", bufs=1, space="PSUM") as pd_pool,
        ):
            ones_sq = cpool.tile([P, P], BF16)
            nc.gpsimd.memset(ones_sq, 1.0)
            # pre-warm the PE during the input-DMA wait so the HAM clock
            # gate is at 2.4 GHz when real work starts (a PE-idle gap
            # > ~3.4us would re-throttle and the first real groups would
            # run at 1.2 GHz)
            warm_db = pd_pool.tile([P, QCH], F32, tag="db", name="warm")
            for wi in range(72):
                nc.tensor.matmul(warm_db[:, ds(0, 64)], ones_sq,
                                 ones_sq[:, :64], start=True, stop=True)

            # input pieces: separate tiles so their DMAs are independent
            # (a single destination tile serializes the piece DMAs WAW).
            blk0_sb = cpool.tile([P, 4 * 512], BF16, name="blk0")
            kp, qp, vp = {}, {}, {}
            kp[0] = blk0_sb[:, ds(0, 512)]
            qp[0] = blk0_sb[:, ds(512, 512)]
            qp[2] = blk0_sb[:, ds(1024, 512)]
            vp[0] = blk0_sb[:, ds(1536, 512)]
            for pi, (c0, c1) in enumerate(KV_PIECES):
                if pi == 0:
                    continue
                kp[pi] = cpool.tile([P, c1 - c0], BF16, name=f"kp{pi}")
                vp[pi] = cpool.tile([P, c1 - c0], BF16, name=f"vp{pi}")
            for pi, (c0, c1) in enumerate(Q_PIECES):
                if pi in (0, 2):
                    continue
                qp[pi] = cpool.tile([P, c1 - c0], BF16, name=f"qp{pi}")
            # ring discipline: scalar ring carries blk0's first 768 cols
            # (k0 + half of q0), sync ring is FIFO [rest of blk0, then
            # pieces in first-use order]
            nc.scalar.dma_start(blk0_sb[:, ds(0, 768)], blk0[:, ds(0, 768)])
            nc.sync.dma_start(blk0_sb[:, ds(768, 1280)],
                              blk0[:, ds(768, 1280)])
            for tname, pi in (("k", 1), ("v", 1), ("q", 3),
                              ("k", 2), ("v", 2), ("q", 4),
                              ("k", 3), ("v", 3), ("q", 1)):
                tbl = Q_PIECES if tname == "q" else KV_PIECES
                c0, c1 = tbl[pi]
                dst, src = {"q": (qp, qT), "k": (kp, kT),
                            "v": (vp, vT)}[tname]
                nc.sync.dma_start(dst[pi], src[:, ds(c0, c1 - c0)])

            def piece_of(table, col):
                for pi, (c0, c1) in enumerate(table):
                    if c0 <= col < c1:
                        return pi, c0
                raise AssertionError(col)

            def k_tile(j):
                pi, c0 = piece_of(KV_PIECES, j * P)
                return kp[pi][:, ds(j * P - c0, P)]

            def v_tile(j):
                pi, c0 = piece_of(KV_PIECES, j * P)
                return vp[pi][:, ds(j * P - c0, P)]

            def q_chunk(t):
                pi, c0 = piece_of(Q_PIECES, t * QCH)
                return qp[pi][:, ds(t * QCH - c0, QCH)]

            o_tiles, db_tiles = {}, {}

            def emit_pv(ops):
                # deferred PV / den matmuls / chunk flushes for one group
                # (software pipelining: keeps the in-order PE queue's S
                # matmuls ahead of PVs that wait on the gpsimd select)
                for op in ops:
                    kind = op[0]
                    if kind == "pv":
                        _, t, pos, j, nj, e_sb, d = op
                        dd = j - 4 * t
                        off = max(dd, 0) * P
                        nc.tensor.matmul(
                            o_tiles[t][:, ds(off, QCH - off)],
                            v_tile(j),
                            e_sb[:, ds(d * QCH + off, QCH - off)],
                            start=(pos == 0), stop=(pos == nj - 1))
                    elif kind == "dwin":
                        # below-diagonal window: ping-pong add chain then
                        # one ones-matmul (partition reduce) into db
                        _, t, slices, st_, sp_ = op
                        acc = slices[0]
                        for sl in slices[1:]:
                            nxt = qpool.tile([P, QCH], BF16, tag="qacc")
                            nc.vector.tensor_add(nxt, acc, sl)
                            acc = nxt
                        nc.tensor.matmul(db_tiles[t], ones_sq, acc,
                                         start=st_, stop=sp_)
                    elif kind == "ddiag":
                        # diagonal window: copy(d0) + narrowed in-place adds
                        _, t, diag, st_ = op
                        dacc = qpool.tile([P, QCH], BF16, tag="qacc")
                        diag = sorted(diag, key=lambda x: x[1])
                        sl0, o0 = diag[0]
                        assert o0 == 0
                        nc.vector.tensor_copy(dacc, sl0)
                        for sl, off in diag[1:]:
                            nc.vector.tensor_add(
                                dacc[:, ds(off, QCH - off)],
                                dacc[:, ds(off, QCH - off)],
                                sl[:, ds(off, QCH - off)])
                        nc.tensor.matmul(db_tiles[t], ones_sq, dacc,
                                         start=st_, stop=True)
                    else:   # flush: copy chunk outputs + DMA out
                        _, t = op
                        o_ps, db_ps = o_tiles[t], db_tiles[t]
                        out_sb = spool.tile([P, QCH], BF16, tag="osb",
                                            name=f"osb{t}")
                        den_sb = spool.tile([1, QCH], F32, tag="den",
                                            name=f"den{t}")
                        if t == CHUNK_ORDER[-1]:   # tail: split engines
                            nc.scalar.copy(out_sb, o_ps)
                            nc.vector.tensor_copy(den_sb, db_ps[0:1, :])
                            nc.sync.dma_start(outT[:, ts(t, QCH)], out_sb)
                            nc.scalar.dma_start(den[:, ts(t, QCH)], den_sb)
                        else:
                            nc.vector.tensor_copy(out_sb, o_ps)
                            nc.vector.tensor_copy(den_sb, db_ps[0:1, :])
                            nc.sync.dma_start(outT[:, ts(t, QCH)], out_sb)
                            nc.sync.dma_start(den[:, ts(t, QCH)], den_sb)

            # per-chunk den bookkeeping (all DVE work deferred with the ops
            # pipeline so Schraudolph TS ops aren't queued behind it):
            #   below-diagonal e-slices collect into windows of DEN_WIN ->
            #   ping-pong add chain + one ones-matmul per window; diagonal
            #   slices are held to the chunk end and summed as
            #   copy(d0-slice) + narrowed in-place adds (no stale prefix).
            den_state = {}   # t -> dict(pend=[slices], nmm, diag=[(sl,off)])

            def den_below(t, sl, ops):
                st = den_state[t]
                st["pend"].append(sl)
                if len(st["pend"]) >= DEN_WIN:
                    ops.append(("dwin", t, st["pend"], st["nmm"] == 0, False))
                    st["nmm"] += 1
                    st["pend"] = []

            def den_diag_flush(t, ops):
                st = den_state[t]
                if st["pend"]:                 # leftover below-window
                    ops.append(("dwin", t, st["pend"], st["nmm"] == 0, False))
                    st["nmm"] += 1
                    st["pend"] = []
                ops.append(("ddiag", t, st["diag"], st["nmm"] == 0))
                st["nmm"] += 1

            # global tile sequence: per-chunk groups with diagonal tiles
            # leading groups (exp prefix-skip) -- see _chunk_seq.
            entries = []
            for t in CHUNK_ORDER:
                nj = 4 * (t + 1)
                for pos, j in enumerate(_chunk_seq(t)):
                    entries.append((t, pos, j, nj))

            groups = []
            i = 0
            while i < len(entries):
                t0 = entries[i][0]
                g = [entries[i]]
                while (len(g) < GROUP and i + len(g) < len(entries)
                       and entries[i + len(g)][0] == t0):
                    g.append(entries[i + len(g)])
                groups.append(g)
                i += len(g)

            pv_pending = []
            pure_idx = {}      # chunk -> running index of pure-below groups
            for members in groups:
                gn = len(members)
                s_ps = ps_pool.tile([P, gn * QCH], F32, tag="s",
                                    padded_shape=[P, GROUP * QCH])
                for d, (t, pos, j, nj) in enumerate(members):
                    dd = j - 4 * t
                    off = max(dd, 0) * P   # fully-masked column prefix
                    nc.tensor.matmul(
                        s_ps[:, ds(d * QCH + off, QCH - off)],
                        k_tile(j), q_chunk(t)[:, ds(off, QCH - off)],
                        start=True, stop=True)
                e_sb = epool.tile([P, gn * QCH], BF16, tag="e",
                                  padded_shape=[P, GROUP * QCH])
                # group leader's masked prefix is at the window start ->
                # skip it in the activation
                t0, _, j0_, _ = members[0]
                off0 = max(j0_ - 4 * t0, 0) * P
                pure_below = (gn == GROUP and
                              all(j < 4 * t for t, _, j, _ in members))
                sch = False
                if pure_below:
                    pi_ = pure_idx.get(t0, 0)
                    pure_idx[t0] = pi_ + 1
                    sch = (t0, pi_) in SCH_GROUPS
                if sch:
                    nc.vector.tensor_scalar(
                        e_sb[:, ds(0, gn * QCH)].bitcast(mybir.dt.int16),
                        s_ps[:, ds(0, gn * QCH)], SCH_K, SCH_B,
                        mybir.AluOpType.mult, mybir.AluOpType.add)
                else:
                    nc.scalar.activation(e_sb[:, ds(off0, gn * QCH - off0)],
                                         s_ps[:, ds(off0, gn * QCH - off0)],
                                         exp_fn, scale=SCALE)

                # causal mask: only the true 128-col diagonal block needs
                # the select (prefix cols are never read downstream)
                for d, (t, pos, j, nj) in enumerate(members):
                    dd = j - 4 * t
                    if dd >= 0:
                        reg = e_sb[:, ds(d * QCH + dd * P, P)]
                        nc.gpsimd.affine_select(
                            out=reg, in_=reg, compare_op=is_ge,
                            fill=0.0, base=0, pattern=[[1, P]],
                            channel_multiplier=-1)

                ops = []
                for d, (t, pos, j, nj) in enumerate(members):
                    if pos == 0:
                        o_tiles[t] = po_pool.tile([P, QCH], F32, tag="o",
                                                  name=f"o{t}")
                        db_tiles[t] = pd_pool.tile([P, QCH], F32, tag="db",
                                                   name=f"db{t}")
                        den_state[t] = {"pend": [], "nmm": 0, "diag": []}
                    ops.append(("pv", t, pos, j, nj, e_sb, d))
                    dd = j - 4 * t
                    sl = e_sb[:, ts(d, QCH)]
                    if dd >= 0:
                        den_state[t]["diag"].append((sl, dd * P))
                    else:
                        den_below(t, sl, ops)
                    if pos == nj - 1:
                        den_diag_flush(t, ops)
                        ops.append(("flush", t))

                if len(pv_pending) >= 2:
                    emit_pv(pv_pending.pop(0))
                pv_pending.append(ops)

            for ops in pv_pending:
                emit_pv(ops)

    nc.compile()
    return nc


def _get_nc():
    if not _nc_cache:
        _nc_cache.append(_build())
    return _nc_cache[0]


def _prep(query, key, value):
    B, H, W, C = query.shape
    CV = value.shape[-1]
    n = H * W
    q = (np.asarray(query, np.float32).reshape(B, n, C).transpose(0, 2, 1)
         .astype(ml_dtypes.bfloat16))
    q = np.ascontiguousarray(q)
    k = np.ascontiguousarray(
        np.asarray(key, np.float32).reshape(B, n, C).transpose(0, 2, 1)
        .astype(ml_dtypes.bfloat16))
    # vT[b, p, 128j+c] = v[b, 128j+p, c]: k-within-tile on partitions, so a
    # [128, 128] SBUF slice is directly the PV weight tile, and the HBM
    # lines are long and contiguous (8 KB per partition row)
    v = (np.asarray(value, np.float32).reshape(B, n // P, P, CV)
         .transpose(0, 2, 1, 3).reshape(B, P, n // P * CV)
         .astype(ml_dtypes.bfloat16))
    v = np.ascontiguousarray(v)
    b0 = np.ascontiguousarray(
        np.concatenate([k[:, :, :512], q[:, :, :512], q[:, :, 1024:1536],
                        v[:, :, :512]], axis=2))
    return q, k, v, b0


def kernel(query, key, value):
    B, H, W, C = query.shape
    CV = value.shape[-1]
    n = H * W
    q, k, v, b0 = _prep(query, key, value)

    nc = _get_nc()
    in_maps = [{"qT": q[b], "kT": k[b], "vT": v[b], "blk0": b0[b]}
               for b in range(B)]
    res = run_bass_kernel_spmd(nc, in_maps, core_ids=list(range(N_CORES)))

    out = np.empty((B, n, CV), np.float32)
    for b in range(B):
        oT = np.asarray(res.results[b]["outT"]).astype(np.float32)
        dn = res.results[b]["den"]           # [1, 4096]
        out[b] = (oT / dn).T
    return out.reshape(B, H, W, CV)
